# revision 22
# baseline (speedup 1.0000x reference)
"""Multi-head attention (B=4, S=2048, D=1024, H=16) on 8 trn2 NeuronCores.

Sharding: tensor-parallel over heads — core c owns heads [2c, 2c+1]
(= model dims [128c, 128c+128)).  Each core computes q/k/v projections for
its head slice (full batch), local attention, and a partial out-projection
against its 128 columns of Wo.  The 8 partial [B*S, D] outputs are summed
on the host (the all-reduce of the Megatron pattern, done at gather time).

Per-core kernel (bf16 matmul operands, fp32 PSUM accumulation):
  xT  [D, B*S]   : x transposed on host so the contraction dim lands on
                   SBUF partitions (avoids on-chip transposes of x).
  qT/kT [128, S] per batch : head-major [2*64, tokens].
  scores_T [k, q] in PSUM -> exp via ScalarE (scale=1/8 fused) -> bf16.
  v_ext [k-tile, 65*2]     : v natural layout (PE-transposed) with a ones
                   column per head => attn@v yields numerator + softmax
                   denominator in one pass.
  division: fast-NR reciprocal of the denom row (fp32), K=1 fp32r matmul
                   broadcast across partitions, DVE multiply.
  out-proj: outT [128, S] (head dims on partitions) @ WoT slice.

Scheduling: Tile's per-engine issue order follows emission order, and PSUM
accumulation groups MUST be contiguous on the PE (interleaving any other
matmul corrupts them / NRT_EXEC_UNIT_UNRECOVERABLE).  The attention exp is
ScalarE-paced, so scores phases leave the PE under-occupied and the HAM
clock-gate re-throttles it to 1.2 GHz.  To keep the PE dense we hand-
interleave self-contained PE work units (projection groups, v-transposes,
attn@v groups of the previous q-chunk, divisions) between the score
matmuls of the current q-chunk — each unit internally contiguous.
"""
import os
import sys

sys.path.insert(0, "/opt/trn_rl_repo")

import numpy as np

import concourse.mybir as mybir
import concourse.tile as tile
from concourse import bacc
from concourse._compat import with_exitstack
from concourse.bass_utils import run_bass_kernel_spmd
from concourse.masks import make_identity
from contextlib import ExitStack

B, S, D, H = 4, 2048, 1024, 16
HD = D // H              # 64
P = 128
NCORES = 8
NH = H // NCORES         # 2 heads per core
T = B * S                # 8192 tokens
DT = D // P              # 8 contraction tiles
KT = S // P              # 16 k-tiles per batch
QC = 1024                # q-chunk (2 psum banks, halves ACT overhead)
NQC = S // QC            # 2
HQ = 512                 # matmul free-dim chunk (one psum bank)
TC = 512                 # projection token chunk
NTC = S // TC            # 4
EXP_SCALE = float(1.0 / np.sqrt(HD))

f32 = mybir.dt.float32
f32r = mybir.dt.float32r
bf16 = mybir.dt.bfloat16

LAST_EXEC_TIME_NS = None
_CACHED_NC = None


@with_exitstack
def _mha_kernel(ctx: ExitStack, tc_: tile.TileContext, ins, outs):
    nc = tc_.nc
    xT_d, wqT_d, wkT_d, wvT_d, woT_d, ones_d = ins
    out_d = outs[0]

    const = ctx.enter_context(tc_.tile_pool(name="const", bufs=1))
    wpool = ctx.enter_context(tc_.tile_pool(name="wpool", bufs=1))
    xpool = ctx.enter_context(tc_.tile_pool(name="xpool", bufs=3))
    qpool = ctx.enter_context(tc_.tile_pool(name="qpool", bufs=1))
    kpool = ctx.enter_context(tc_.tile_pool(name="kpool", bufs=1))
    vpool = ctx.enter_context(tc_.tile_pool(name="vpool", bufs=1))
    vxpool = ctx.enter_context(tc_.tile_pool(name="vxpool", bufs=1))
    epool = ctx.enter_context(tc_.tile_pool(name="epool", bufs=48))
    opool = ctx.enter_context(tc_.tile_pool(name="opool", bufs=1))
    rpool = ctx.enter_context(tc_.tile_pool(name="rpool", bufs=2))
    ospool = ctx.enter_context(tc_.tile_pool(name="ospool", bufs=2))

    # single PSUM pool: 4 rotating [128,1024] slots (2 banks each)
    psum = ctx.enter_context(tc_.tile_pool(name="psum", bufs=4, space="PSUM"))

    ident = const.tile([P, P], f32, tag="ident")
    make_identity(nc, ident)
    ones_b = const.tile([P, 2], bf16, tag="ones_b")
    nc.gpsimd.dma_start(ones_b[:], ones_d[:, 0:2])
    ones_r = const.tile([1, HD], f32r, tag="ones_r")
    nc.sync.dma_start(ones_r[:], ones_d[0:1, 2:2 + HD].bitcast(f32r))

    wq = wpool.tile([P, D], bf16, tag="wq")
    wk = wpool.tile([P, D], bf16, tag="wk")
    wv = wpool.tile([P, D], bf16, tag="wv")
    wo = wpool.tile([P, D], bf16, tag="wo")
    for w_sb, w_d in ((wq, wqT_d), (wk, wkT_d), (wv, wvT_d)):
        # [D, 128] DRAM -> [128, DT*128] SBUF, d-tile major, cast to bf16
        nc.gpsimd.dma_start(
            w_sb[:].rearrange("p (dt o) -> p dt o", dt=DT),
            w_d[:].rearrange("(dt p) o -> p dt o", p=P),
        )
    nc.gpsimd.dma_start(wo[:], woT_d[:])

    xT_r = xT_d[:].rearrange("(dt p) t -> p dt t", p=P)

    xts = {}

    def load_x(t, lb):
        xt = xpool.tile([P, DT * TC], bf16, tag="xt", name=f"xt_{lb}_{t}")
        nc.gpsimd.dma_start(
            xt[:].rearrange("p (dt t) -> p dt t", dt=DT),
            xT_r[:, :, lb * S + t * TC: lb * S + (t + 1) * TC],
        )
        xts[(lb, t)] = xt

    for b in range(B):
        qT = qpool.tile([P, S], bf16, tag="qT")
        kT = kpool.tile([P, S], bf16, tag="kT")
        vT = vpool.tile([P, S], f32, tag="vT")
        v_ext = vxpool.tile([P, KT * 2 * (HD + 1)], bf16, tag="vext")
        outT = opool.tile([P, S], bf16, tag="outT")


        def proj(w_sb, dstT, t):
            # one contiguous 8-matmul accumulation group + eviction
            pp = psum.tile([P, QC], f32, tag="ps")
            for dt in range(DT):
                nc.tensor.matmul(
                    pp[:, 0:TC],
                    w_sb[:, dt * P:(dt + 1) * P],
                    xts[(b, t)][:, dt * TC:(dt + 1) * TC],
                    start=(dt == 0), stop=(dt == DT - 1),
                )
            nc.vector.tensor_copy(dstT[:, t * TC:(t + 1) * TC], pp[:, 0:TC])

        def vtrans(kt):
            vps = psum.tile([P, QC], f32, tag="ps")
            nc.tensor.transpose(
                vps[:, 0:P], vT[:, kt * P:(kt + 1) * P], ident[:])
            base = kt * 2 * (HD + 1)
            nc.vector.tensor_copy(v_ext[:, base:base + HD], vps[:, 0:HD])
            nc.vector.tensor_copy(
                v_ext[:, base + HD + 1:base + 2 * HD + 1], vps[:, HD:2 * HD])
            nc.vector.tensor_copy(v_ext[:, base + HD:base + HD + 1],
                                  ones_b[:, 0:1])
            nc.vector.tensor_copy(
                v_ext[:, base + 2 * HD + 1:base + 2 * HD + 2], ones_b[:, 1:2])

        def scores(qc, kt, exp_tiles):
            for h in range(NH):
                hs = slice(h * HD, (h + 1) * HD)
                sc = psum.tile([P, QC], f32, tag="ps")
                for half in range(QC // HQ):
                    nc.tensor.matmul(
                        sc[:, half * HQ:(half + 1) * HQ],
                        kT[hs, kt * P:(kt + 1) * P],
                        qT[hs, qc * QC + half * HQ: qc * QC + (half + 1) * HQ],
                        start=True, stop=True,
                    )
                ex = epool.tile([P, QC], bf16, tag="exp")
                nc.scalar.activation(
                    ex[:], sc[:], mybir.ActivationFunctionType.Exp,
                    scale=EXP_SCALE)
                exp_tiles[h][kt] = ex

        def attnv_group(oes, h, half, exp_tiles):
            # one contiguous 16-matmul accumulation group
            if oes[h] is None:
                oes[h] = psum.tile([P, QC], f32, tag="ps", name=f"oe_h{h}")
            hq = slice(half * HQ, (half + 1) * HQ)
            for kt in range(KT):
                base = kt * 2 * (HD + 1) + h * (HD + 1)
                nc.tensor.matmul(
                    oes[h][0:HD + 1, hq],
                    v_ext[:, base:base + HD + 1],
                    exp_tiles[h][kt][:, hq],
                    start=(kt == 0), stop=(kt == KT - 1),
                )

        def div_pre(oes, h, recrs):
            # DVE-only reciprocal chain; emit as early as possible
            oe = oes[h]
            den = rpool.tile([1, QC], f32, tag="den")
            # custom-DVE ops mis-read partition-offset inputs: stage the
            # denominator row to partition 0 first
            nc.vector.tensor_copy(den[:], oe[HD:HD + 1, :])
            rec = rpool.tile([1, QC], f32, tag="rec")
            scr = rpool.tile([1, QC], f32, tag="scr")
            nc.vector.reciprocal_approx_accurate(rec[:], den[:], scr[:])
            recr = rpool.tile([1, QC], f32r, tag="recr")
            with nc.allow_low_precision(reason="feeds bcast matmul"):
                nc.vector.tensor_copy(recr[:], rec[:])
            recrs[h] = recr

        def div_post(oes, qc, h, recrs):
            oe = oes[h]
            qs = slice(qc * QC, (qc + 1) * QC)
            recr = recrs[h]
            rb_ps = psum.tile([P, QC], f32, tag="ps")
            for half in range(QC // HQ):
                hq = slice(half * HQ, (half + 1) * HQ)
                nc.tensor.matmul(rb_ps[0:HD, hq], ones_r[:], recr[:, hq],
                                 start=True, stop=True)
            rb = rpool.tile([HD, QC], f32, tag="rb")
            nc.vector.tensor_copy(rb[:], rb_ps[0:HD, :])
            nc.vector.tensor_mul(outT[h * HD:(h + 1) * HD, qs],
                                 oe[0:HD, :], rb[:])

        def oproj(t):
            po = psum.tile([P, QC], f32, tag="ps")
            for ec in range(D // HQ):
                nc.tensor.matmul(
                    po[:, ec * HQ:(ec + 1) * HQ],
                    outT[:, t * P:(t + 1) * P],
                    wo[:, ec * HQ:(ec + 1) * HQ],
                    start=True, stop=True,
                )
            osb = ospool.tile([P, D], f32, tag="osb")
            # alternate eviction engine so the DVE doesn't back up
            if t % 2 == 0:
                nc.vector.tensor_copy(osb[:], po[:])
            else:
                nc.scalar.copy(osb[:], po[:])
            nc.sync.dma_start(
                out_d[b * S + t * P: b * S + (t + 1) * P, :],
                osb[:],
            )

        # ---------- batch schedule ----------
        # prologue: x chunks 0-1, k/q/v projections 0-1, v-transposes 0-7
        if (b, 0) not in xts:
            load_x(0, b)
        if (b, 1) not in xts:
            load_x(1, b)
        for t in (0, 1):
            proj(wk, kT, t)
            proj(wq, qT, t)
            proj(wv, vT, t)
        load_x(2, b)
        for kt in (0, 1, 2, 3):
            vtrans(kt)
        for kt in (4, 5, 6, 7):
            vtrans(kt)

        exp0 = [[None] * KT for _ in range(NH)]
        # qc0 scores with the remaining projections/transposes as PE filler
        fillers = [
            lambda: proj(wk, kT, 2),
            lambda: (load_x(3, b), proj(wk, kT, 3)),
            lambda: proj(wq, qT, 2),
            lambda: proj(wv, vT, 2),
            lambda: (vtrans(8), vtrans(9)),
            lambda: proj(wq, qT, 3),
            lambda: proj(wv, vT, 3),
            lambda: (vtrans(10), vtrans(11)),
            lambda: (vtrans(12), vtrans(13)),
            lambda: (vtrans(14), vtrans(15)),
        ]
        for kt in range(KT):
            scores(0, kt, exp0)
            if fillers:
                fillers.pop(0)()
            if kt >= 6 and fillers:
                fillers.pop(0)()

        # qc1 scores with qc0's attn@v groups + divisions as PE filler
        exp1 = [[None] * KT for _ in range(NH)]
        oes0 = [None, None]
        recrs0 = [None, None]
        fillers = [
            lambda: attnv_group(oes0, 0, 0, exp0),
            lambda: attnv_group(oes0, 0, 1, exp0),
            lambda: (div_pre(oes0, 0, recrs0), attnv_group(oes0, 1, 0, exp0)),
            lambda: div_post(oes0, 0, 0, recrs0),
            lambda: (attnv_group(oes0, 1, 1, exp0),
                     div_pre(oes0, 1, recrs0),
                     load_x(0, b + 1) if b + 1 < B else None),
            lambda: (div_post(oes0, 0, 1, recrs0),
                     load_x(1, b + 1) if b + 1 < B else None),
        ] + [(lambda tt: lambda: oproj(tt))(t_) for t_ in range(8)]
        for kt in range(KT):
            scores(1, kt, exp1)
            if fillers:
                fillers.pop(0)()
            if kt >= 9 and fillers:
                fillers.pop(0)()
        for f in fillers:
            f()

        # flush qc1 attn@v + divisions, interleaved with out-projection
        oes1 = [None, None]
        recrs1 = [None, None]
        attnv_group(oes1, 0, 0, exp1)
        attnv_group(oes1, 0, 1, exp1)
        div_pre(oes1, 0, recrs1)
        attnv_group(oes1, 1, 0, exp1)
        div_post(oes1, 1, 0, recrs1)
        attnv_group(oes1, 1, 1, exp1)
        div_pre(oes1, 1, recrs1)
        div_post(oes1, 1, 1, recrs1)
        for t in range(8, S // P):
            oproj(t)


def _build():
    global _CACHED_NC
    if _CACHED_NC is not None:
        return _CACHED_NC
    nc = bacc.Bacc("TRN2", target_bir_lowering=False, debug=False)
    xT = nc.dram_tensor("xT", [D, T], f32, kind="ExternalInput").ap()
    wqT = nc.dram_tensor("wqT", [D, P], f32, kind="ExternalInput").ap()
    wkT = nc.dram_tensor("wkT", [D, P], f32, kind="ExternalInput").ap()
    wvT = nc.dram_tensor("wvT", [D, P], f32, kind="ExternalInput").ap()
    woT = nc.dram_tensor("woT", [P, D], f32, kind="ExternalInput").ap()
    ones = nc.dram_tensor("ones", [P, HD + 2], f32, kind="ExternalInput").ap()
    out = nc.dram_tensor("out", [T, D], f32, kind="ExternalOutput").ap()

    with tile.TileContext(nc) as tc_:
        _mha_kernel(tc_, [xT, wqT, wkT, wvT, woT, ones], [out])
    nc.compile()
    _CACHED_NC = nc
    return nc


def kernel(x: np.ndarray, Wq: np.ndarray, Wk: np.ndarray, Wv: np.ndarray,
           Wo: np.ndarray) -> np.ndarray:
    global LAST_EXEC_TIME_NS
    nc = _build()

    x = np.asarray(x, dtype=np.float32)
    xT = np.ascontiguousarray(x.reshape(T, D).T)          # [D, T]
    ones_in = np.ones((P, HD + 2), dtype=np.float32)

    in_maps = []
    for c in range(NCORES):
        rows = slice(c * P, (c + 1) * P)
        in_maps.append({
            "xT": xT,
            "wqT": np.ascontiguousarray(np.asarray(Wq, np.float32)[rows, :].T),
            "wkT": np.ascontiguousarray(np.asarray(Wk, np.float32)[rows, :].T),
            "wvT": np.ascontiguousarray(np.asarray(Wv, np.float32)[rows, :].T),
            "woT": np.ascontiguousarray(np.asarray(Wo, np.float32)[:, rows].T),
            "ones": ones_in,
        })

    trace = bool(os.environ.get("BASS_TRACE"))
    res = run_bass_kernel_spmd(nc, in_maps, core_ids=list(range(NCORES)),
                               trace=trace)
    LAST_EXEC_TIME_NS = res.exec_time_ns

    acc = res.results[0]["out"].astype(np.float32)
    for c in range(1, NCORES):
        acc = acc + res.results[c]["out"]
    return acc.reshape(B, S, D)
